# revision 1
# baseline (speedup 1.0000x reference)
"""Channel-selective 1x1-conv MLP + per-pixel sorted top-k for TRN2 (8 NeuronCores).

Reference computation (per pixel p, channels c=0..255):
    h   = w1 @ x[:,p] + b1                  (256 -> 128)
    x_  = w2 @ h + b2                       (128 -> 256)
    xi  = indices of top-128 of sigmoid(x_) (desc order)
    out[k,p] = x[xi_k,p] * x_[xi_k,p]

Since sigmoid is strictly monotone, top-k indices of sigmoid(x_) == top-k
indices of x_ itself, so the sigmoid is never computed.

V2 strategy (vs V1 which used max/max_index/match_replace = 47 DVE insts/tile):
 - The channel index is EMBEDDED in the low 8 mantissa bits of each fp32 key
   ((bits & ~0xFF) | c). DVE max8 rounds then return value+index fused, so
   the 16 MaxIndex instructions per tile disappear. All keys become distinct,
   making match_replace collision-free.
 - The truncation perturbs ordering for near-ties (~2^-16 relative); a
   2-layer odd-even compare-exchange cleanup using the EXACT fp32 keys
   (gathered via two int16-plane local_scatters) restores the reference
   order except for a negligible tail (validated: rel L2 ~5e-3).
 - Per tile DVE: mul + AND + OR embed + 16 max8 + 15 match_replace + idx
   extract + 9 narrow cleanup ops; Pool (GpSimd) does the scatters and
   casts, ACT the copies, PE the GEMMs/transposes — all hidden under the
   DVE stream by a 4-deep cross-engine software pipeline (every engine
   queue is in-order, so each stage's cross-engine inputs are produced a
   full rounds-duration before consumption).

Measured floor: a pure-DVE microbench (dve_only=True) shows each 256-wide
max8/match_replace costs ~642 ns on HW (2x the cost model, engine-
intrinsic); the full kernel runs within ~8% of that 31-instruction floor.
Perturbations that LOST same-epoch A/B ratio tests (kept as default-off
build flags for re-testing): half_rounds, interleave (pair-interleaved
round chains), pad_maxv (64B-aligned max8 outputs), pingpong (out-of-place
match_replace). Bench note: consecutive device benches vary +-30-45%; only
best-of-N or alternating A/B ratios are meaningful.
"""

import numpy as np

import concourse.bass as bass
import concourse.mybir as mybir
from concourse import bacc
from concourse.tile import TileContext
from concourse.masks import make_identity

B, C, H, W = 16, 256, 80, 80
MED, K = 128, 128
HWP = H * W            # 6400 pixels per image
NCORES = 8
BPC = B // NCORES      # images per core
P = 128                # pixels per tile (SBUF partitions)
COLS = HWP // P        # 50 column tiles per image
NT = BPC * COLS        # 100 tiles per core
NEG = -1.0e30          # match_replace fill, below any reachable key

F32 = mybir.dt.float32
F16 = mybir.dt.float16
I32 = mybir.dt.int32
I16 = mybir.dt.int16
U16 = mybir.dt.uint16
U8 = mybir.dt.uint8
ACT_COPY = mybir.ActivationFunctionType.Copy
ALU = mybir.AluOpType


def build(
    n_tiles: int = NT,
    repeat: int = 1,
    no_topk: bool = False,
    no_cleanup: bool = False,
    no_scatter: bool = False,
    no_tail: bool = False,
    half_rounds: bool = False,
    interleave: bool = False,
    pad_maxv: bool = False,
    pingpong: bool = False,
    dve_only: bool = False,
) -> "bacc.Bacc":
    nc = bacc.Bacc(None, target_bir_lowering=False, debug=True)
    x_ext = nc.declare_dram_parameter("x", [BPC, C, HWP], F32, isOutput=False)
    w1_ext = nc.declare_dram_parameter("w1", [MED, C], F32, isOutput=False)
    b1_ext = nc.declare_dram_parameter("b1", [1, MED], F32, isOutput=False)
    w2_ext = nc.declare_dram_parameter("w2", [C, MED], F32, isOutput=False)
    b2_ext = nc.declare_dram_parameter("b2", [1, C], F32, isOutput=False)
    out_ext = nc.declare_dram_parameter("out", [BPC, K, HWP], F32, isOutput=True)

    with TileContext(nc) as tc:
        with (
            tc.tile_pool(name="const", bufs=1) as cpool,
            tc.tile_pool(name="io", bufs=4) as iop,
            tc.tile_pool(name="wk", bufs=2) as wk,
            tc.tile_pool(name="psum", bufs=2, space="PSUM") as pp,
        ):
            # ---------------- constants ----------------
            ident = cpool.tile([P, P], F32)
            make_identity(nc, ident)

            ones_row = cpool.tile([1, P], F32)
            nc.vector.memset(ones_row, 1.0)

            # iota via lower-tri matmul (avoids gpsimd.iota / library games):
            # iotap[p, i] = i+1 (f32)
            ones_sq = cpool.tile([P, P], F32)
            nc.vector.memset(ones_sq, 1.0)
            tri = cpool.tile([P, P], F32)
            nc.gpsimd.affine_select(
                out=tri,
                in_=ones_sq,
                compare_op=ALU.is_ge,
                fill=0.0,
                base=0,
                pattern=[[1, P]],
                channel_multiplier=-1,
            )
            iotap = pp.tile([P, K], F32, tag="tr")
            nc.tensor.matmul(iotap, lhsT=ones_sq, rhs=tri, start=True, stop=True)
            iota1 = cpool.tile([P, K], I16)  # each partition: 1..128
            nc.scalar.copy(iota1, iotap)
            # iota_bits[p, c] = c (int32), c = 0..255
            iota_bits = cpool.tile([P, C], I32)
            nc.scalar.activation(iota_bits[:, 0:K], iotap, ACT_COPY, bias=-1.0)
            nc.scalar.activation(iota_bits[:, K:C], iotap, ACT_COPY, bias=127.0)

            if dve_only:
                # pure-DVE microbench: per "tile", one 256-wide copy + the
                # 31-instruction max8/match_replace chain, nothing else.
                kc = cpool.tile([P, C], F32)
                nc.scalar.activation(kc[:, 0:K], iotap, ACT_COPY, scale=1.0)
                nc.scalar.activation(kc[:, K:C], iotap, ACT_COPY, bias=128.0)

                def dve_tile(t):
                    kemb = wk.tile([P, C], F32, tag="kemb", bufs=3)
                    nc.vector.tensor_copy(kemb, kc)
                    maxv = wk.tile([P, K], F32, tag="maxv", bufs=3)
                    for r in range(16):
                        sl = slice(8 * r, 8 * r + 8)
                        nc.vector.max(out=maxv[:, sl], in_=kemb)
                        if r < 15:
                            nc.vector.match_replace(
                                out=kemb, in_to_replace=maxv[:, sl],
                                in_values=kemb, imm_value=NEG,
                            )
                    col = (t % COLS) * P
                    nc.sync.dma_start(
                        out=out_ext[t // COLS, :, col : col + P], in_=maxv
                    )

                if repeat == 1:
                    for t in range(n_tiles):
                        dve_tile(t)
                else:
                    with tc.For_i(0, repeat, 1):
                        for t in range(n_tiles):
                            dve_tile(t)
                return nc

            from concourse import library_config
            nc.gpsimd.load_library(library_config.local_scatter)

            b1row = cpool.tile([1, MED], F32)
            nc.sync.dma_start(out=b1row, in_=b1_ext[:, :])
            b2row = cpool.tile([1, C], F32)
            nc.sync.dma_start(out=b2row, in_=b2_ext[:, :])

            # w1T: [256(c), 128(m)] as two [128,128] tiles
            w1sb = iop.tile([MED, C], F32, tag="wload")
            nc.sync.dma_start(out=w1sb, in_=w1_ext[:, :])
            w1T = []
            for h in range(2):
                tp = pp.tile([P, P], F32, tag="tr")
                nc.tensor.transpose(tp, w1sb[:, P * h : P * (h + 1)], ident)
                wt = cpool.tile([P, MED], F32, name=f"w1T{h}")
                nc.scalar.copy(wt, tp)
                w1T.append(wt)

            # w2T: [128(m), 256(c)] single tile
            w2T = cpool.tile([MED, C], F32)
            for h in range(2):
                w2sb = iop.tile([P, MED], F32, tag="wload2")
                nc.sync.dma_start(out=w2sb, in_=w2_ext[P * h : P * (h + 1), :])
                tp = pp.tile([P, P], F32, tag="tr")
                nc.tensor.transpose(tp, w2sb, ident)
                nc.scalar.copy(w2T[:, P * h : P * (h + 1)], tp)

            # ---------------- main loop ----------------
            # 4-deep software pipeline. All engines (DVE/Pool/ACT/PE) execute
            # their queues IN ORDER, so each stage's cross-engine inputs are
            # produced >= 1 full rounds-duration before they are consumed:
            #   stA(t):   DMA+GEMMs+copies (PE/ACT), embed-prep (Pool),
            #             mul+OR+16x max8 rounds (DVE)
            #   stB(t-1): idx extract + all 4 local_scatters (Pool)
            #   stC(t-2): g32 interleave + q2 base (ACT), cleanup L1 (DVE)
            #   stD(t-3): cleanup L2 (DVE), transpose (PE) + store
            def stA(t):
                b, j = divmod(t, COLS)
                col = j * P
                st = {"b": b, "col": col}

                x0 = iop.tile([P, P], F32, tag="x0")
                nc.sync.dma_start(out=x0, in_=x_ext[b, 0:P, col : col + P])
                x1 = iop.tile([P, P], F32, tag="x1")
                nc.sync.dma_start(out=x1, in_=x_ext[b, P:C, col : col + P])

                # GEMM1: h[m, n] += w1T.T @ x  (+ b1 via rank-1 matmul)
                hp = pp.tile([P, P], F32, tag="h")
                nc.tensor.matmul(hp, lhsT=w1T[0], rhs=x0, start=True, stop=False)
                nc.tensor.matmul(hp, lhsT=w1T[1], rhs=x1, start=False, stop=False)
                nc.tensor.matmul(hp, lhsT=b1row, rhs=ones_row, start=False, stop=True)
                hs = wk.tile([P, P], F32, tag="hs", bufs=3)
                nc.scalar.copy(hs, hp)

                # GEMM2 (pixel-major): x_T[n, c] = h.T @ w2T (+ b2 broadcast)
                xtp = pp.tile([P, C], F32, tag="xt")
                nc.tensor.matmul(xtp, lhsT=hs, rhs=w2T, start=True, stop=False)
                nc.tensor.matmul(xtp, lhsT=ones_row, rhs=b2row, start=False, stop=True)
                keys = wk.tile([P, C], F32, tag="keys", bufs=4)
                nc.scalar.copy(keys, xtp)

                # payload: x transposed to pixel-major (f32)
                xTf = wk.tile([P, C], F32, tag="xTf", bufs=3)
                for h, xh in enumerate((x0, x1)):
                    tp = pp.tile([P, P], F32, tag="tr")
                    nc.tensor.transpose(tp, xh, ident)
                    nc.scalar.copy(xTf[:, P * h : P * (h + 1)], tp)

                # Pool-side prep: exact-key int16 planes
                keys16 = keys.bitcast(I16)
                lo_pl = wk.tile([P, C], I16, tag="lo_pl", bufs=4)
                nc.gpsimd.tensor_scalar(
                    lo_pl, keys16[:, 0 : 2 * C : 2], 0, None, ALU.bypass,
                )
                hi_pl = wk.tile([P, C], I16, tag="hi_pl", bufs=4)
                nc.gpsimd.tensor_scalar(
                    hi_pl, keys16[:, 1 : 2 * C : 2], 0, None, ALU.bypass,
                )
                st["lo_pl"], st["hi_pl"] = lo_pl, hi_pl

                st["keys"], st["xTf"] = keys, xTf
                return st

            def stA_dve(sts):
                # DVE work for 1-2 tiles. With two tiles the max8/match_replace
                # chains are INTERLEAVED so each instruction's input was
                # produced 2 instructions earlier, hiding the write-to-read
                # turnaround of the strictly serial per-tile chain.
                for st in sts:
                    p16 = wk.tile([P, C], F16, tag="p16", bufs=6)
                    nc.vector.tensor_mul(p16, st["keys"], st["xTf"])
                    st["p16"] = p16
                for st in sts:
                    emb_a = wk.tile([P, C], F32, tag="emb_a", bufs=3)
                    nc.vector.tensor_scalar(
                        emb_a.bitcast(I32), st["keys"].bitcast(I32),
                        -256, None, ALU.bitwise_and,  # 0xFFFFFF00 as int32
                    )
                    st["emb_a"] = emb_a
                for st in sts:
                    kemb = wk.tile([P, C], F32, tag="kemb", bufs=3)
                    nc.vector.tensor_tensor(
                        kemb.bitcast(I32), st["emb_a"].bitcast(I32), iota_bits,
                        ALU.bitwise_or,
                    )
                    st["kemb"] = kemb[:, 0:128] if half_rounds else kemb
                    # pad_maxv: each round's 8-f32 output starts its own 64B
                    # SBUF line (stride 16) to avoid partial-line RMW between
                    # consecutive rounds
                    stm = 16 if pad_maxv else 8
                    maxv = wk.tile([P, 16 * stm], F32, tag="maxv", bufs=4)
                    st["maxv"] = maxv
                    st["stm"] = stm
                if pingpong:
                    # match_replace writes a SECOND buffer instead of
                    # rewriting kemb in place (avoids same-region
                    # read+write streams within one instruction)
                    for st in sts:
                        kemb2 = wk.tile([P, C], F32, tag="kemb2", bufs=3)
                        st["kemb2"] = kemb2
                if no_topk:
                    for st in sts:
                        nc.vector.tensor_copy(st["maxv"][:, 0:K], st["kemb"][:, 0:K])
                else:
                    for r in range(16):
                        for st in sts:
                            sl = slice(st["stm"] * r, st["stm"] * r + 8)
                            nc.vector.max(out=st["maxv"][:, sl], in_=st["kemb"])
                        if r < 15:
                            for st in sts:
                                sl = slice(st["stm"] * r, st["stm"] * r + 8)
                                dst = st["kemb2"] if pingpong else st["kemb"]
                                nc.vector.match_replace(
                                    out=dst,
                                    in_to_replace=st["maxv"][:, sl],
                                    in_values=st["kemb"],
                                    imm_value=NEG,
                                )
                                if pingpong:
                                    st["kemb"], st["kemb2"] = (
                                        st["kemb2"], st["kemb"],
                                    )
                # channel idx of each sorted slot = low 8 bits of the key
                # (DVE: Pool rejects bitwise ops)
                for st in sts:
                    mv = st["maxv"].bitcast(I32)
                    if st["stm"] != 8:
                        mv = mv.rearrange("p (g s) -> p g s", g=16)[:, :, 0:8]
                    else:
                        mv = mv[:, 0:K]
                    idx32 = wk.tile([P, K], I32, tag="idx32", bufs=6)
                    nc.vector.tensor_scalar(
                        idx32, mv, 0xFF, None, ALU.bitwise_and,
                    )
                    st["idx32"] = idx32

            def stB(st):
                if no_topk or no_scatter:
                    return
                p16 = st["p16"]
                # cast idx to i16 for the scatter (arith ops may cast on Pool)
                idxu = wk.tile([P, K], I16, tag="idxu", bufs=6)
                nc.gpsimd.tensor_scalar(idxu, st["idx32"], 0, None, ALU.add)
                # rank inversion: rankp1[c] = slot+1 for selected c, else 0
                rankp1 = wk.tile([P, C], I16, tag="rankp1", bufs=4)
                nc.gpsimd.local_scatter(
                    rankp1, iota1, idxu,
                    channels=P, num_elems=C, num_idxs=K,
                )
                ranks = wk.tile([P, C], I16, tag="ranks", bufs=6)
                nc.gpsimd.tensor_scalar(ranks, rankp1, -1, None, ALU.add)

                # gather products + exact-key planes into sorted slot order
                q16 = wk.tile([P, K], F16, tag="q16", bufs=6)
                nc.gpsimd.local_scatter(
                    q16, p16, ranks,
                    channels=P, num_elems=K, num_idxs=C,
                )
                st["q16"] = q16
                if no_cleanup:
                    return
                glo = wk.tile([P, K], I16, tag="glo", bufs=4)
                nc.gpsimd.local_scatter(
                    glo, st["lo_pl"], ranks,
                    channels=P, num_elems=K, num_idxs=C,
                )
                ghi = wk.tile([P, K], I16, tag="ghi", bufs=4)
                nc.gpsimd.local_scatter(
                    ghi, st["hi_pl"], ranks,
                    channels=P, num_elems=K, num_idxs=C,
                )
                st["glo"], st["ghi"] = glo, ghi

            def stC(st):
                if no_topk or no_scatter or no_cleanup:
                    return
                q16 = st["q16"]
                # re-interleave exact keys into one f32 tile (ACT)
                g32 = wk.tile([P, K], F32, tag="g32", bufs=4)
                g16v = g32.bitcast(I16)
                nc.scalar.copy(g16v[:, 0 : 2 * K : 2], st["glo"])
                nc.scalar.copy(g16v[:, 1 : 2 * K : 2], st["ghi"])
                q2 = wk.tile([P, K], F16, tag="q2", bufs=4)
                nc.scalar.copy(q2, q16)

                # cleanup layer 1: pairs (0,1),(2,3),... (DVE)
                ge = g32[:, 0:K:2]
                go = g32[:, 1:K:2]
                m1 = wk.tile([P, K // 2], U8, tag="m1", bufs=4)
                nc.vector.tensor_tensor(m1, ge, go, ALU.is_lt)
                g2 = wk.tile([P, K], F32, tag="g2", bufs=4)
                nc.vector.tensor_tensor(g2[:, 0:K:2], ge, go, ALU.max)
                nc.vector.tensor_tensor(g2[:, 1:K:2], ge, go, ALU.min)
                nc.vector.copy_predicated(q2[:, 0:K:2], m1, q16[:, 1:K:2])
                nc.vector.copy_predicated(q2[:, 1:K:2], m1, q16[:, 0:K:2])
                st["g2"], st["q2"] = g2, q2

            def stD(st):
                b, col = st["b"], st["col"]
                prod = wk.tile([P, K], F32, tag="prod", bufs=4)
                if no_topk or no_scatter:
                    nc.scalar.copy(prod, st["p16"][:, 0:K])
                elif no_cleanup:
                    nc.scalar.copy(prod, st["q16"])
                else:
                    g2, q2 = st["g2"], st["q2"]
                    # cleanup layer 2: pairs (1,2),(3,4),...,(125,126) (DVE)
                    g2e = g2[:, 1 : K - 1 : 2]
                    g2o = g2[:, 2 : K - 1 : 2]
                    m2 = wk.tile([P, K // 2 - 1], U8, tag="m2", bufs=4)
                    nc.vector.tensor_tensor(m2, g2e, g2o, ALU.is_lt)
                    nc.vector.tensor_copy(prod, q2)
                    nc.vector.copy_predicated(
                        prod[:, 1 : K - 1 : 2], m2, q2[:, 2 : K - 1 : 2]
                    )
                    nc.vector.copy_predicated(
                        prod[:, 2 : K - 1 : 2], m2, q2[:, 1 : K - 1 : 2]
                    )

                if no_tail:
                    # bench-only: store untransposed
                    nc.sync.dma_start(out=out_ext[b, :, col : col + P], in_=prod)
                    return
                # transpose back to [k, n] and store
                op = pp.tile([P, P], F32, tag="otr")
                nc.tensor.transpose(op, prod, ident)
                osb = wk.tile([P, P], F32, tag="osb", bufs=4)
                nc.scalar.copy(osb, op)
                nc.sync.dma_start(out=out_ext[b, :, col : col + P], in_=osb)

            def loop_body():
                ring = {}
                step = 2 if interleave else 1
                groups = [
                    list(range(i, min(i + step, n_tiles)))
                    for i in range(0, n_tiles, step)
                ]
                ng = len(groups)
                for i in range(ng + 3):
                    if i < ng:
                        sts = [stA(t) for t in groups[i]]
                        stA_dve(sts)
                        ring[i] = sts
                    if 1 <= i and i - 1 in ring:
                        for s in ring[i - 1]:
                            stB(s)
                    if 2 <= i and i - 2 in ring:
                        for s in ring[i - 2]:
                            stC(s)
                    if 3 <= i and i - 3 in ring:
                        for s in ring.pop(i - 3):
                            stD(s)

            if repeat == 1:
                loop_body()
            else:
                with tc.For_i(0, repeat, 1):
                    loop_body()

    return nc


def _run(inputs, trace: bool = False):
    from concourse.bass_utils import run_bass_kernel_spmd

    x = np.ascontiguousarray(inputs["x"], dtype=np.float32).reshape(B, C, HWP)
    w1 = np.ascontiguousarray(inputs["w1"], dtype=np.float32)
    b1 = np.ascontiguousarray(inputs["b1"], dtype=np.float32).reshape(1, MED)
    w2 = np.ascontiguousarray(inputs["w2"], dtype=np.float32)
    b2 = np.ascontiguousarray(inputs["b2"], dtype=np.float32).reshape(1, C)
    assert int(inputs.get("out_c", K)) == K

    nc = build()
    nc.finalize()  # runs the Bacc passes (reg alloc, ISA codegen, lib loads)
    core_ids = list(range(NCORES))
    in_maps = [
        {
            "x": np.ascontiguousarray(x[i * BPC : (i + 1) * BPC]),
            "w1": w1,
            "b1": b1,
            "w2": w2,
            "b2": b2,
        }
        for i in core_ids
    ]
    res = None
    for attempt in range(3):
        try:
            res = run_bass_kernel_spmd(nc, in_maps, core_ids, trace=trace)
            break
        except Exception:
            # rare transient NRT_EXEC_UNIT_UNRECOVERABLE device hiccups;
            # the NEFF is compile-cached so a retry is cheap
            if attempt == 2:
                raise
    out = np.concatenate([r["out"] for r in res.results], axis=0)
    return out.reshape(B, K, H, W), res


def kernel(**inputs) -> np.ndarray:
    out, _ = _run(inputs, trace=False)
    return out


if __name__ == "__main__":
    # tiny smoke test of the builder only
    nc = build(n_tiles=1)
    print("build ok:", nc)

